# revision 1
# baseline (speedup 1.0000x reference)
"""YOLO-style loss kernel for Trainium2, 8-core data-parallel.

Strategy:
  - Shard batch (1024) as 128 per NeuronCore (pure data parallelism).
  - Host repacks each core's shard into 35 channel-planes laid out
    [128 partitions(batch), 35 planes, 784 cells] fp16 (validated to ~5e-5
    relative error vs the f32 reference), halving HBM traffic; all on-chip
    elementwise math runs at the DVE's 2x fp16 rate.
  - Key algebra: the grid offsets (gi, gj) cancel inside the IoU, and the
    whole loss is a sum of squares of masked per-cell values, so the
    device reduces everything with fused Square+accumulate ACT ops into
    one [128,1] partial per core; the host sums 8x128 partials and
    divides by the batch size.

Units: boxes are handled in grid-cell units (IoU is scale invariant):
  half-extent = 14*w (w*IMG_SIZE/GRID_SIZE = 28*w is the full extent);
  areas enter the denominator as 784*(wa*ha + wt*ht) to match the
  intersection's cell^2 scale. 1/x is computed as exp(-ln(x+eps)).
"""

import numpy as np

from concourse import bacc, mybir, tile
from concourse.bass_utils import run_bass_kernel_spmd

F32 = mybir.dt.float32
F16 = mybir.dt.float16
OP = mybir.AluOpType
AF = mybir.ActivationFunctionType

B, S, NCLS = 1024, 28, 20
NCORES = 8
BP = B // NCORES          # 128 batches per core = 128 partitions
CELLS = S * S             # 784
NPL = 15 + NCLS           # 35 planes
EPS = 1e-4                # denominator guard, fp16-safe (ref uses 1e-12)
SQ5 = float(np.sqrt(5.0))
SQH = float(np.sqrt(0.5))
NQ = 4                    # class planes processed in quarters of 5
QP = NCLS // NQ

# plane indices in the packed input
T0, AX, CX, TX, AY, CY, TY = 0, 1, 2, 3, 4, 5, 6
AW, CW, TW, AH, CH, TH = 7, 8, 9, 10, 11, 12
P4, P9 = 13, 14
K0 = 15                   # 20 class planes [15, 35)

_CACHED = None


def _build_kernel():
    nc = bacc.Bacc(None, target_bir_lowering=False)
    planes = nc.dram_tensor("planes", [BP, NPL, CELLS], F16, kind="ExternalInput")
    partials = nc.dram_tensor("partials", [BP, 1], F32, kind="ExternalOutput")

    with tile.TileContext(nc) as tc:
        with (
            tc.tile_pool(name="inp", bufs=1) as inp,
            tc.tile_pool(name="wk", bufs=1) as wk,
            tc.tile_pool(name="rot", bufs=2) as rot,
            tc.tile_pool(name="cin", bufs=3) as cin,
        ):
            # ---- load group A: t0, xy, wh, confs (15 planes) -------------
            a15 = inp.tile([BP, 15, CELLS], F16, tag="a15")
            nc.sync.dma_start(a15[:], planes[:, 0:15, :])
            # class planes: quarters of 5, loaded while group-A math runs
            cls_t = []
            for q in range(NQ):
                ct_ = cin.tile([BP, QP, CELLS], F16, tag="clsin")
                nc.sync.dma_start(
                    ct_[:], planes[:, K0 + q * QP : K0 + (q + 1) * QP, :]
                )
                cls_t.append(ct_)

            xy = a15[:, AX : TY + 1, :]            # [ax,cx,tx, ay,cy,ty]
            wh = a15[:, AW : TH + 1, :]            # [aw,cw,tw, ah,ch,th]
            xy4 = xy.rearrange("p (g c) s -> p g c s", g=2)  # [:, xy, (a,c,t), :]

            # ---- corners (negated lo): LO' = 14*wh - xy ; HI = xy + 14*wh
            lo = wk.tile([BP, 6, CELLS], F16)
            hi = wk.tile([BP, 6, CELLS], F16)
            nc.vector.scalar_tensor_tensor(lo[:], wh, 14.0, xy, OP.mult, OP.subtract)
            nc.vector.scalar_tensor_tensor(hi[:], wh, 14.0, xy, OP.mult, OP.add)

            # ---- raw areas [pa, pc, pt] ---------------------------------
            ar = wk.tile([BP, 3, CELLS], F16)
            nc.gpsimd.tensor_tensor(ar[:], wh[:, 0:3, :], wh[:, 3:6, :], OP.mult)

            # ---- intersection: iw = relu(min(hi) + min(lo')) ------------
            lo4 = lo[:].rearrange("p (g c) s -> p g c s", g=2)
            hi4 = hi[:].rearrange("p (g c) s -> p g c s", g=2)
            tb = (BP, 2, 2, CELLS)
            minl = wk.tile([BP, 2, 2, CELLS], F16)
            minh = wk.tile([BP, 2, 2, CELLS], F16)
            nc.vector.tensor_tensor(
                minl[:], lo4[:, :, 0:2, :], lo4[:, :, 2:3, :].broadcast_to(tb), OP.min
            )
            nc.vector.tensor_tensor(
                minh[:], hi4[:, :, 0:2, :], hi4[:, :, 2:3, :].broadcast_to(tb), OP.min
            )
            d = wk.tile([BP, 2, 2, CELLS], F16)
            nc.vector.tensor_tensor(d[:], minh[:], minl[:], OP.add)
            dr = wk.tile([BP, 2, 2, CELLS], F16)
            nc.scalar.activation(dr[:], d[:], AF.Relu)

            itr = wk.tile([BP, 2, CELLS], F16)    # [interA, interC]
            nc.vector.tensor_tensor(itr[:], dr[:, 0, :, :], dr[:, 1, :, :], OP.mult)

            # ---- denominator: 784*(p + pt) - inter ----------------------
            s2 = wk.tile([BP, 2, CELLS], F16)
            nc.gpsimd.tensor_tensor(
                s2[:], ar[:, 0:2, :], ar[:, 2:3, :].broadcast_to((BP, 2, CELLS)), OP.add
            )
            den = wk.tile([BP, 2, CELLS], F16)
            nc.vector.scalar_tensor_tensor(
                den[:], s2[:], 784.0, itr[:], OP.mult, OP.subtract
            )

            # ---- iou = inter * exp(-ln(den + eps)) ----------------------
            eps_t = wk.tile([BP, 1], F32)
            nc.vector.memset(eps_t[:], EPS)
            lnd = wk.tile([BP, 2, CELLS], F32)
            nc.scalar.activation(lnd[:], den[:], AF.Ln, bias=eps_t[:])
            rcp = wk.tile([BP, 2, CELLS], F16)
            nc.scalar.activation(rcp[:], lnd[:], AF.Exp, scale=-1.0)
            iou = wk.tile([BP, 2, CELLS], F16)
            nc.vector.tensor_tensor(iou[:], itr[:], rcp[:], OP.mult)

            iouA, iouC = iou[:, 0:1, :], iou[:, 1:2, :]

            # ---- box choice ---------------------------------------------
            m = wk.tile([BP, 1, CELLS], F16)
            nc.vector.tensor_tensor(m[:], iouA, iouC, OP.is_gt)
            ct = wk.tile([BP, 1, CELLS], F16)
            nc.vector.tensor_tensor(ct[:], iouA, iouC, OP.max)

            # conf_pred: blend cp = p9 + m*(p4 - p9)
            cp = wk.tile([BP, 1, CELLS], F16)
            nc.vector.tensor_tensor(
                cp[:], a15[:, P4 : P4 + 1, :], a15[:, P9 : P9 + 1, :], OP.subtract
            )
            nc.vector.tensor_tensor(cp[:], m[:], cp[:], OP.mult)
            nc.vector.tensor_tensor(cp[:], cp[:], a15[:, P9 : P9 + 1, :], OP.add)

            # xy_sel = cxy + m*(axy - cxy)
            xysel = wk.tile([BP, 2, 1, CELLS], F16)
            mb = m[:].unsqueeze(1).broadcast_to((BP, 2, 1, CELLS))
            nc.vector.tensor_tensor(
                xysel[:], xy4[:, :, 0:1, :], xy4[:, :, 1:2, :], OP.subtract
            )
            nc.vector.tensor_tensor(xysel[:], mb, xysel[:], OP.mult)
            nc.vector.tensor_tensor(xysel[:], xysel[:], xy4[:, :, 1:2, :], OP.add)

            # ---- masks ---------------------------------------------------
            mobj = wk.tile([BP, 1, CELLS], F16)
            nc.vector.tensor_scalar(mobj[:], a15[:, T0 : T0 + 1, :], 0.0, None, OP.is_gt)
            mobj5 = wk.tile([BP, 1, CELLS], F16)
            nc.vector.tensor_scalar(mobj5[:], mobj[:], SQ5, None, OP.mult)
            nm = wk.tile([BP, 1, CELLS], F16)        # sqrt(.5)*(1-mobj)
            nc.vector.tensor_scalar(nm[:], mobj[:], -SQH, SQH, OP.mult, OP.add)

            # ---- small masked pieces block v5: [me, mex, mey, n4, n9] ---
            v5 = wk.tile([BP, 5, CELLS], F16)
            e = wk.tile([BP, 1, CELLS], F16)
            nc.vector.tensor_tensor(e[:], cp[:], ct[:], OP.subtract)
            nc.vector.tensor_tensor(v5[:, 0:1, :], mobj[:], e[:], OP.mult)
            exy = wk.tile([BP, 2, 1, CELLS], F16)
            nc.vector.tensor_tensor(exy[:], xysel[:], xy4[:, :, 2:3, :], OP.subtract)
            nc.vector.tensor_tensor(
                v5[:, 1:3, :],
                mobj5[:].broadcast_to((BP, 2, CELLS)),
                exy[:].rearrange("p a o s -> p (a o) s"),
                OP.mult,
            )
            nc.vector.tensor_tensor(
                v5[:, 3:5, :],
                nm[:].broadcast_to((BP, 2, CELLS)),
                a15[:, P4 : P9 + 1, :],
                OP.mult,
            )

            acc = wk.tile([BP, 1 + NQ], F32)
            scr5 = rot.tile([BP, QP, CELLS], F16, tag="scr")
            nc.scalar.activation(scr5[:], v5[:], AF.Square, accum_out=acc[:, 0:1])

            # ---- class block, quarters of 5 planes ----------------------
            for q in range(NQ):
                oh = rot.tile([BP, QP, CELLS], F16, tag="oh")
                for k in range(QP):
                    nc.gpsimd.tensor_scalar(
                        oh[:, k : k + 1, :],
                        a15[:, T0 : T0 + 1, :],
                        float(q * QP + k + 1),
                        None,
                        OP.is_equal,
                    )
                mp = rot.tile([BP, QP, CELLS], F16, tag="mp")
                nc.vector.tensor_tensor(
                    mp[:], mobj[:].broadcast_to((BP, QP, CELLS)), cls_t[q][:], OP.mult
                )
                nc.vector.tensor_tensor(mp[:], mp[:], oh[:], OP.subtract)
                scr = rot.tile([BP, QP, CELLS], F16, tag="scr")
                nc.scalar.activation(
                    scr[:], mp[:], AF.Square, accum_out=acc[:, 1 + q : 2 + q]
                )

            # ---- finalize: partial[p] = sum(acc[p, :]) ------------------
            out_sb = wk.tile([BP, 1], F32)
            nc.vector.tensor_reduce(
                out_sb[:], acc[:], axis=mybir.AxisListType.X, op=OP.add
            )
            nc.sync.dma_start(partials[:], out_sb[:])

    nc.compile()
    return nc


def _pack(y_pred, y_true):
    """[1024,28,28,30]+[1024,28,28,5] -> [8, 128, 35, 784] float16."""
    yp = y_pred.reshape(NCORES, BP, CELLS, 30).transpose(0, 1, 3, 2)
    yt = y_true.reshape(NCORES, BP, CELLS, 5).transpose(0, 1, 3, 2)
    out = np.empty((NCORES, BP, NPL, CELLS), dtype=np.float16)
    out[:, :, T0] = yt[:, :, 0]
    out[:, :, AX] = yp[:, :, 0]
    out[:, :, CX] = yp[:, :, 5]
    out[:, :, TX] = yt[:, :, 1]
    out[:, :, AY] = yp[:, :, 1]
    out[:, :, CY] = yp[:, :, 6]
    out[:, :, TY] = yt[:, :, 2]
    out[:, :, AW] = yp[:, :, 2]
    out[:, :, CW] = yp[:, :, 7]
    out[:, :, TW] = yt[:, :, 3]
    out[:, :, AH] = yp[:, :, 3]
    out[:, :, CH] = yp[:, :, 8]
    out[:, :, TH] = yt[:, :, 4]
    out[:, :, P4] = yp[:, :, 4]
    out[:, :, P9] = yp[:, :, 9]
    out[:, :, K0 : K0 + 20] = yp[:, :, 10:30]
    return np.ascontiguousarray(out)


def kernel(y_pred: np.ndarray, y_true: np.ndarray, _trace=False) -> np.ndarray:
    global _CACHED
    if _CACHED is None:
        _CACHED = _build_kernel()
    nc = _CACHED
    packed = _pack(np.asarray(y_pred, np.float32), np.asarray(y_true, np.float32))
    in_maps = [{"planes": packed[c]} for c in range(NCORES)]
    res = run_bass_kernel_spmd(nc, in_maps, core_ids=list(range(NCORES)), trace=_trace)
    kernel.last_result = res
    total = np.float64(0.0)
    for c in range(NCORES):
        total += np.asarray(res.results[c]["partials"], np.float64).sum()
    return np.float32(total / B)

